# revision 1
# baseline (speedup 1.0000x reference)
"""DualCrossAttention Trainium2 kernel.

Data-parallel: batch=8 across 8 NeuronCores, one batch element per core.
Per core: two cross-attentions + FFN + 3 LayerNorms on [768, 512] activations.

Layout: feature-major activations (x.T: [feature(part), seq(free)]); weights
host-pre-transposed so every projection is a plain PE matmul. Attention uses
S.T = k_h @ q_h.T with the two heads of a pair emitted as adjacent matmuls
into PE row-groups 0/64 (concurrent execution); exp on ACT with the wm scale
folded in; O.T accumulated with a ones-augmented V column so the softmax
denominator lands in PSUM row 64; normalizers are broadcast across partitions
via a DRAM-bounce DMA. All matmul operands are float32r.
"""
import contextlib

import numpy as np

import concourse.bacc as bacc
import concourse.bass as bass
import concourse.tile as tile
from concourse import mybir
from concourse.bass_utils import run_bass_kernel_spmd
import concourse.bass_utils as _bu

# walrus elides redundant LDWEIGHTS (adjacent region matmuls share lhsT)
# when ldw-opt is on; concourse pins it off, flip just that flag.
if not getattr(_bu, "_ldwopt_patched", False):
    _orig_run_command = _bu.run_command

    def _run_command_ldwopt(argv, **kw):
        argv = ["--enable-ldw-opt=true" if a == "--enable-ldw-opt=false"
                else a for a in argv]
        return _orig_run_command(argv, **kw)

    _bu.run_command = _run_command_ldwopt
    _bu._ldwopt_patched = True

F32 = mybir.dt.float32
F32R = mybir.dt.float32r
AF = mybir.ActivationFunctionType
ALU = mybir.AluOpType

H, KD, VD = 8, 64, 64
D, DF = 512, 2048
S1, S2, S3 = 768, 1024, 768
P = 128
NCH = D // P            # 4 feature chunks of the 512-dim residual stream
W = 1.25                # wm weight scale
INV_SQRT = 0.125        # 1/sqrt(64)
EPS = 1e-5

_PROGRAM_CACHE = {}


def _regions(n):
    """Split free dim n into <=512 column regions (PSUM-bank aligned)."""
    out = []
    s = 0
    while s < n:
        e = min(s + 512, n)
        out.append((s, e))
        s = e
    return out


class _Ctx:
    """Shared handles for the emit helpers."""
    pass


def _emit_proj(nc, psum_pool, wT_sb, xT_sb, n_out, n_seq, k_chunks, consume):
    """out.T[o, i] = sum_d wT[d, o] * xT[d, i]; calls consume(m, psum_ap)."""
    for m in range(n_out // P):
        ps = psum_pool.tile([P, 1024], F32, tag="proj", name=f"pp{m}")
        # ko-major: both region matmuls of a (m, ko) pair are consecutive and
        # share their lhsT, so walrus ldw-opt elides every second LDWEIGHTS
        for ko in range(k_chunks):
            for (a, b) in _regions(n_seq):
                nc.tensor.matmul(
                    ps[:, a:b],
                    wT_sb[:, ko, m * P:(m + 1) * P],
                    xT_sb[:, ko, a:b],
                    start=(ko == 0), stop=(ko == k_chunks - 1),
                )
        consume(m, ps[:, :n_seq])


def _emit_ln(nc, cx, psum_pool, z_src, resid, bias_pm, y_sb, n_seq, scale_gb):
    """LayerNorm over the feature axis (partitions x NCH chunks).

    z_src(m) -> psum AP [P, n_seq] (projection output chunk m);
    z = psum + bias + resid is built in y_sb and normalized in place.
    Stats via ones-matmuls (partition reduction on PE); rstd via
    exp(-0.5 ln(var+eps)); mean/rstd broadcast via DRAM bounce.
    """
    sb, dram = cx.sb, cx.dram
    z_sb = y_sb
    # f32r matmul dst must start at partition 0: one PSUM tile per stat row
    stat_z = psum_pool.tile([1, 768], F32, tag="ln_stat_z", bufs=1)
    stat_zsq = psum_pool.tile([1, 768], F32, tag="ln_stat_zsq", bufs=1)
    ones_z = cx.ones_sb if z_sb.dtype == F32R else cx.ones_f32
    for m in range(NCH):
        ps = z_src(m)
        if bias_pm is not None:
            nc.vector.scalar_tensor_tensor(
                z_sb[:, m, :], ps, bias_pm[:, m:m + 1], resid[:, m, :],
                op0=ALU.add, op1=ALU.add)
        else:
            nc.vector.tensor_tensor(z_sb[:, m, :], ps, resid[:, m, :], ALU.add)
        zsq = cx.zsq_pool.tile([P, 768], F32R, tag="ln_zsq", name=f"zsq{m}")
        nc.gpsimd.tensor_tensor(zsq[:, :n_seq], z_sb[:, m, :], z_sb[:, m, :],
                                ALU.mult)
        for (a, b) in _regions(n_seq):
            nc.tensor.matmul(stat_z[0:1, a:b], ones_z[:, 0:1],
                             z_sb[:, m, a:b],
                             start=(m == 0), stop=(m == NCH - 1))
            nc.tensor.matmul(stat_zsq[0:1, a:b], cx.ones_sb[:, 0:1],
                             zsq[:, a:b],
                             start=(m == 0), stop=(m == NCH - 1))
    # small per-position vectors: engine partition starts must be 32-aligned,
    # keep each on its own partition-0 tile
    mean_t = sb.tile([1, n_seq], F32, tag="ln_mean")
    nc.vector.tensor_scalar_mul(mean_t[:], stat_z[0:1, :n_seq], 1.0 / D)
    msq = sb.tile([1, n_seq], F32, tag="ln_msq")
    nc.vector.tensor_tensor(msq[:], mean_t[:], mean_t[:], ALU.mult)
    rstd_t = sb.tile([1, n_seq], F32, tag="ln_rstd")
    mr_t = sb.tile([1, n_seq], F32, tag="ln_mr")
    # var = sum(z^2)/D - mean^2 -> ln(var+eps) -> rstd = exp(-0.5*ln)
    nc.vector.scalar_tensor_tensor(rstd_t[:], stat_zsq[0:1, :n_seq], 1.0 / D,
                                   msq[:], op0=ALU.mult, op1=ALU.subtract)
    nc.scalar.activation(rstd_t[:], rstd_t[:], AF.Ln, bias=cx.eps_sb[0:1, :],
                         scale=1.0)
    nc.scalar.activation(rstd_t[:], rstd_t[:], AF.Exp, bias=0.0, scale=-0.5)
    nc.vector.tensor_tensor(mr_t[:], mean_t[:], rstd_t[:], ALU.mult)
    dln = dram.tile([2, n_seq], F32, tag="ln_dram")
    nc.sync.dma_start(dln[0:1, :], rstd_t[:])
    nc.sync.dma_start(dln[1:2, :], mr_t[:])
    rstd_bc = sb.tile([P, n_seq], F32, tag="ln_rstd_bc")
    mr_bc = sb.tile([P, n_seq], F32, tag="ln_mr_bc")
    nc.gpsimd.dma_start(rstd_bc[:], dln[0:1, :].to_broadcast([P, n_seq]))
    nc.gpsimd.dma_start(mr_bc[:], dln[1:2, :].to_broadcast([P, n_seq]))
    for m in range(NCH):
        nc.vector.tensor_tensor(y_sb[:, m, :], y_sb[:, m, :], rstd_bc[:],
                                ALU.mult)
        nc.vector.tensor_tensor(y_sb[:, m, :], y_sb[:, m, :], mr_bc[:],
                                ALU.subtract)
        if scale_gb is not None:
            g_sb, b_sb = scale_gb
            nc.vector.tensor_scalar(
                y_sb[:, m, :], y_sb[:, m, :],
                g_sb[:, m:m + 1], b_sb[:, m:m + 1], op0=ALU.mult, op1=ALU.add)


def _exp_slices(layer, j, n_q):
    """Per (attention layer, key-chunk j): (col_lo, col_hi, exp scale)."""
    if layer == 1:
        # wm1: (i<512, j<512) and (i>=512, j>=512) get W
        jlo = j * P < 512
        s_lo = W * INV_SQRT if jlo else INV_SQRT
        s_hi = INV_SQRT if jlo else W * INV_SQRT
        return [(0, 512, s_lo), (512, n_q, s_hi)]
    # wm2: diagonal 256-blocks get W; key chunk j lies in block j//2;
    # adjacent equal-scale ranges merged to cut ACT op count
    blk = j // 2
    raw = [(b * 256, min((b + 1) * 256, n_q),
            W * INV_SQRT if b == blk else INV_SQRT) for b in range(3)]
    out = [raw[0]]
    for (lo, hi, sc) in raw[1:]:
        plo, phi, psc = out[-1]
        if sc == psc and lo == phi:
            out[-1] = (plo, hi, sc)
        else:
            out.append((lo, hi, sc))
    return out


def _emit_attn(nc, cx, work, psum_s, psum_ot, qT_sb, kT_sb, v_sb, ot_sb,
               layer, n_q, n_kv):
    """Cross-attention. Per (key-chunk, column-region) unit: the two heads of
    a pair are emitted as ADJACENT K=64 matmuls (PE row-groups 0/64 execute
    them concurrently); exp on ACT with the wm scale folded in; O.T
    accumulated per unit with a ones-augmented V column so the softmax
    denominator lands in PSUM row 64. S for unit u+1 is emitted before O of
    unit u so the PE has independent work while ACT computes exp."""
    sb, dram = cx.sb, cx.dram
    J = n_kv // P
    heads = lambda c: ((slice(0, 64), 2 * c), (slice(64, 128), 2 * c + 1))
    units = [(j, a, b) for j in range(J) for (a, b) in _regions(n_q)]

    def emit_S_unit(c, u, etiles):
        (hb_e, h_e), (hb_o, h_o) = heads(c)
        j, a, b = units[u]
        if j not in etiles:
            etiles[j] = (
                work.tile([P, n_q], F32R, tag="exps", bufs=4,
                          name=f"ee{c}_{j}"),
                work.tile([P, n_q], F32R, tag="exps", bufs=4,
                          name=f"eo{c}_{j}"),
            )
        e_e, e_o = etiles[j]
        ps_e = psum_s.tile([P, 512], F32, tag="s", name=f"se{c}_{j}_{a}")
        ps_o = psum_s.tile([P, 512], F32, tag="s", name=f"so{c}_{j}_{a}")
        nc.tensor.matmul(ps_e[:, :b - a],
                         kT_sb[hb_e, c, j * P:(j + 1) * P],
                         qT_sb[hb_e, c, a:b], start=True, stop=True)
        nc.tensor.matmul(ps_o[:, :b - a],
                         kT_sb[hb_o, c, j * P:(j + 1) * P],
                         qT_sb[hb_o, c, a:b], start=True, stop=True)
        for e, ps in ((e_e, ps_e), (e_o, ps_o)):
            for (lo, hi, sc) in _exp_slices(layer, j, n_q):
                lo2, hi2 = max(lo, a), min(hi, b)
                if lo2 < hi2:
                    nc.scalar.activation(
                        e[:, lo2:hi2], ps[:, lo2 - a:hi2 - a],
                        AF.Exp, bias=0.0, scale=sc)

    def emit_O_unit(c, u, etiles, po_all):
        j, a, b = units[u]
        e_e, e_o = etiles[j]
        for (hb, h), e in zip(heads(c), (e_e, e_o)):
            nc.tensor.matmul(
                po_all[h][0:65, a:b],
                v_sb[:, j, h, 0:65],
                e[:, a:b],
                start=(j == 0), stop=(j == J - 1))

    for c in range(H // 2):
        po_all = {}
        for hb, h in heads(c):
            po_all[h] = psum_ot.tile([65, 768], F32, tag="ot", name=f"po{h}")
        etiles = {}
        emit_S_unit(c, 0, etiles)
        for u in range(len(units)):
            if u + 1 < len(units):
                emit_S_unit(c, u + 1, etiles)
            emit_O_unit(c, u, etiles, po_all)
        drp = dram.tile([2, n_q], F32, tag="drp", name=f"drp{c}")
        for idx, (hb, h) in enumerate(heads(c)):
            # raw O.T out of PSUM (frees banks); colsum row staged at
            # partition 0 (engine copies cannot write partition starts 1..31)
            nc.vector.tensor_copy(ot_sb[hb, c, :], po_all[h][0:64, :n_q])
            srow = work.tile([1, n_q], F32, tag="sumrow", bufs=2,
                             name=f"srow{h}")
            nc.vector.tensor_copy(srow[:], po_all[h][64:65, :n_q])
            rrow = work.tile([1, n_q], F32, tag="rrow", bufs=2,
                             name=f"rrow{h}")
            nc.vector.reciprocal_approx_fast(out=rrow[:], in_=srow[:])
            nc.sync.dma_start(drp[idx:idx + 1, :], rrow[:])
        bc = work.tile([P, n_q], F32, tag="attn_bc", name=f"bc{c}")
        nc.gpsimd.dma_start(bc[0:64, :], drp[0:1, :].to_broadcast([64, n_q]))
        nc.gpsimd.dma_start(bc[64:128, :],
                            drp[1:2, :].to_broadcast([64, n_q]))
        nc.vector.tensor_tensor(ot_sb[:, c, :], ot_sb[:, c, :], bc[:],
                                ALU.mult)


def _r3(ap):
    """DRAM [K*128, n] -> [128(part), K, n] view for DMA."""
    return ap.rearrange("(ko p) s -> p ko s", p=P)


def _build_program(flags):
    use_bo1, use_bo2, use_fb1, use_fb2, use_g1, use_g2, use_g3 = flags
    nc = bacc.Bacc("TRN2", target_bir_lowering=False, debug=False)

    def din(name, shape, dt=F32R):
        return nc.dram_tensor(name, shape, dt, kind="ExternalInput").ap()

    x1T = din("x1T", [D, S1])
    x2T = din("x2T", [D, S2])
    x3T = din("x3T", [D, S3])
    wts = {n: din(n, [D, D]) for n in
           ("wq1T", "wk1T", "wv1T", "wo1T", "wq2T", "wk2T", "wv2T", "wo2T")}
    fw1T = din("fw1T", [D, DF])
    fw2T = din("fw2T", [DF, D])
    onesd = din("onesd", [P, 1])
    vones = din("vones", [P, H])
    bo1 = din("bo1", [P, NCH], F32) if use_bo1 else None
    bo2 = din("bo2", [P, NCH], F32) if use_bo2 else None
    fb1 = din("fb1", [P, DF // P], F32) if use_fb1 else None
    fb2 = din("fb2", [P, NCH], F32) if use_fb2 else None
    gbd = {}
    for i, use in ((1, use_g1), (2, use_g2), (3, use_g3)):
        gbd[i] = (din(f"g{i}", [P, NCH], F32),
                  din(f"b{i}", [P, NCH], F32)) if use else None
    yT = nc.dram_tensor("yT", [D, S1], F32, kind="ExternalOutput").ap()

    with tile.TileContext(nc, pool_alloc_mode="queue") as tc:
        cx = _Ctx()
        cx.tc = tc
        with tc.tile_pool(name="sb", bufs=1) as sb, \
             tc.tile_pool(name="zsq", bufs=1) as zsq_pool, \
             tc.tile_pool(name="dram", bufs=2, space="DRAM") as dram:
            cx.sb, cx.dram, cx.zsq_pool = sb, dram, zsq_pool

            ones_sb = sb.tile([P, 1], F32R, tag="ones")
            nc.sync.dma_start(ones_sb[:], onesd)
            cx.ones_sb = ones_sb
            eps_sb = sb.tile([P, 1], F32, tag="eps")
            nc.vector.memset(eps_sb[:], EPS)
            cx.eps_sb = eps_sb
            ones_f32 = sb.tile([P, 2], F32, tag="ones_f32")
            nc.vector.memset(ones_f32[:], 1.0)
            cx.ones_f32 = ones_f32[:, 0:1]

            def load_pm(ap, cols, tag):
                if ap is None:
                    return None
                t = sb.tile([P, cols], F32, tag=tag)
                nc.sync.dma_start(t[:], ap)
                return t

            bo1_sb = load_pm(bo1, NCH, "bo1")
            bo2_sb = load_pm(bo2, NCH, "bo2")
            fb1_sb = load_pm(fb1, DF // P, "fb1")
            fb2_sb = load_pm(fb2, NCH, "fb2")
            gb_sb = {}
            for i in (1, 2, 3):
                gb_sb[i] = None if gbd[i] is None else (
                    load_pm(gbd[i][0], NCH, f"g{i}"),
                    load_pm(gbd[i][1], NCH, f"b{i}"))

            y1_sb = sb.tile([P, NCH, S1], F32R, tag="y1")
            y2_sb = sb.tile([P, NCH, S1], F32R, tag="y2")

            def copy_cb(dst):
                return lambda m, ps: nc.vector.tensor_copy(dst[:, m, :], ps)

            def emit_v_proj(psum_pool, x_sb, wv_sb, v_sb, Jkv):
                for j in range(Jkv):
                    nc.sync.dma_start(v_sb[:, j, :, 64:65], vones[:, :, None])
                    ps = psum_pool.tile([P, 1024], F32, tag="proj",
                                        name=f"vps{j}")
                    for ko in range(NCH):
                        nc.tensor.matmul(
                            ps[:, 0:D],
                            x_sb[:, ko, j * P:(j + 1) * P],
                            wv_sb[:, ko, :],
                            start=(ko == 0), stop=(ko == NCH - 1))
                    nc.vector.tensor_copy(
                        v_sb[:, j, :, 0:64],
                        ps[:, 0:D].rearrange("p (h v) -> p h v", h=H))

            # open order is reverse of close order (pool stack is LIFO)
            kv2 = tc.tile_pool(name="kv2", bufs=1)
            with kv2 as kv2p:
                x3_sb = kv2p.tile([P, NCH, S3], F32R, tag="xkv")
                wk2_sb = kv2p.tile([P, NCH, D], F32R, tag="wk")
                wv2_sb = kv2p.tile([P, NCH, D], F32R, tag="wv")
                k2_sb = kv2p.tile([P, NCH, S3], F32R, tag="k")
                v2_sb = kv2p.tile([P, S3 // P, H, 65], F32R, tag="v")

                otp1 = contextlib.ExitStack()
                otp1p = otp1.enter_context(tc.tile_pool(name="otp1", bufs=1))
                x1_sb = otp1p.tile([P, NCH, S1], F32R, tag="x1")
                nc.sync.dma_start(x1_sb[:], _r3(x1T))
                ot_sb = otp1p.tile([P, NCH, S1], F32R, tag="ot1")

                at1_ctx = contextlib.ExitStack()
                at1p = at1_ctx.enter_context(tc.tile_pool(name="at1", bufs=1))
                q_sb = at1p.tile([P, NCH, S1], F32R, tag="q")
                k_sb = at1p.tile([P, NCH, S2], F32R, tag="k")
                v_sb = at1p.tile([P, S2 // P, H, 65], F32R, tag="v")

                kv1_ctx = contextlib.ExitStack()
                kv1p = kv1_ctx.enter_context(tc.tile_pool(name="kv1", bufs=1))
                wq_sb = kv1p.tile([P, NCH, D], F32R, tag="wq")
                nc.sync.dma_start(wq_sb[:], _r3(wts["wq1T"]))
                x2_sb = kv1p.tile([P, NCH, S2], F32R, tag="xkv")
                nc.sync.dma_start(x2_sb[:], _r3(x2T))
                wk_sb = kv1p.tile([P, NCH, D], F32R, tag="wk")
                wv_sb = kv1p.tile([P, NCH, D], F32R, tag="wv")
                nc.sync.dma_start(wk_sb[:], _r3(wts["wk1T"]))
                nc.sync.dma_start(wv_sb[:], _r3(wts["wv1T"]))
                nc.sync.dma_start(x3_sb[:], _r3(x3T))
                nc.sync.dma_start(wk2_sb[:], _r3(wts["wk2T"]))
                nc.sync.dma_start(wv2_sb[:], _r3(wts["wv2T"]))

                # dense warm-up block: q1,k1,v1 then k2,v2
                with tc.tile_pool(name="psA", bufs=3, space="PSUM") as psA:
                    _emit_proj(nc, psA, wq_sb, x1_sb, D, S1, NCH,
                               copy_cb(q_sb))
                    _emit_proj(nc, psA, wk_sb, x2_sb, D, S2, NCH,
                               copy_cb(k_sb))
                    emit_v_proj(psA, x2_sb, wv_sb, v_sb, S2 // P)
                    _emit_proj(nc, psA, wk2_sb, x3_sb, D, S3, NCH,
                               copy_cb(k2_sb))
                    emit_v_proj(psA, x3_sb, wv2_sb, v2_sb, S3 // P)
                kv1_ctx.close()  # frees x2 + wq1/wk1/wv1 SBUF

                # attention 1
                with tc.tile_pool(name="wk1w", bufs=3) as work, \
                     tc.tile_pool(name="ps_s1", bufs=4, space="PSUM") as pss, \
                     tc.tile_pool(name="ps_ot1", bufs=2,
                                  space="PSUM") as psot:
                    _emit_attn(nc, cx, work, pss, psot, q_sb, k_sb, v_sb,
                               ot_sb, 1, S1, S2)
                at1_ctx.close()  # frees q1/k1/v1 SBUF

                # wo1 + LN1
                with tc.tile_pool(name="pwo1", bufs=1) as pwo, \
                     tc.tile_pool(name="psB1", bufs=2, space="PSUM") as psB:
                    wo_sb = pwo.tile([P, NCH, D], F32R, tag="wo")
                    nc.sync.dma_start(wo_sb[:], _r3(wts["wo1T"]))
                    wo_ps = []
                    _emit_proj(nc, psB, wo_sb, ot_sb, D, S1, NCH,
                               lambda m, ps: wo_ps.append(ps))
                    _emit_ln(nc, cx, psB, lambda m: wo_ps[m], x1_sb,
                             bo1_sb, y1_sb, S1, gb_sb[1])
                otp1.close()

                # q2 projection (needs y1)
                otp2 = contextlib.ExitStack()
                otp2p = otp2.enter_context(tc.tile_pool(name="otp2", bufs=1))
                ot2_sb = otp2p.tile([P, NCH, S1], F32R, tag="ot2")
                q2_sb = otp2p.tile([P, NCH, S1], F32R, tag="q2")
                pq2 = contextlib.ExitStack()
                pq2p = pq2.enter_context(tc.tile_pool(name="pq2", bufs=1))
                wq2_sb = pq2p.tile([P, NCH, D], F32R, tag="wq2")
                nc.sync.dma_start(wq2_sb[:], _r3(wts["wq2T"]))
                with tc.tile_pool(name="psC", bufs=3, space="PSUM") as psC:
                    _emit_proj(nc, psC, wq2_sb, y1_sb, D, S1, NCH,
                               copy_cb(q2_sb))
                pq2.close()

                # attention 2
                with tc.tile_pool(name="wk2w", bufs=3) as work2, \
                     tc.tile_pool(name="ps_s2", bufs=4, space="PSUM") as pss2, \
                     tc.tile_pool(name="ps_ot2", bufs=2,
                                  space="PSUM") as psot2:
                    _emit_attn(nc, cx, work2, pss2, psot2, q2_sb, k2_sb,
                               v2_sb, ot2_sb, 2, S1, S3)

                # wo2 + LN2
                with tc.tile_pool(name="pwo2", bufs=1) as pwo2, \
                     tc.tile_pool(name="psD", bufs=2, space="PSUM") as psD:
                    wo2_sb = pwo2.tile([P, NCH, D], F32R, tag="wo")
                    nc.sync.dma_start(wo2_sb[:], _r3(wts["wo2T"]))
                    wo2_ps = []
                    _emit_proj(nc, psD, wo2_sb, ot2_sb, D, S1, NCH,
                               lambda m, ps: wo2_ps.append(ps))
                    _emit_ln(nc, cx, psD, lambda m: wo2_ps[m], y1_sb,
                             bo2_sb, y2_sb, S1, gb_sb[2])
                otp2.close()

            # FFN + LN3
            yT_sb = sb.tile([P, NCH, S1], F32, tag="y1")  # reuse y1 slot
            with tc.tile_pool(name="ffn1", bufs=1) as f1p:
                fw1_sb = f1p.tile([P, NCH, DF], F32R, tag="fw1")
                nc.sync.dma_start(fw1_sb[:], _r3(fw1T))
                h_sb = f1p.tile([P, DF // P, S1], F32R, tag="hT")
                with tc.tile_pool(name="psE", bufs=3, space="PSUM") as psE:
                    def gelu_consume(m, ps):
                        for (a, b) in _regions(S1):
                            nc.scalar.activation(
                                h_sb[:, m, a:b], ps[:, a:b], AF.Gelu,
                                bias=(fb1_sb[:, m:m + 1]
                                      if fb1_sb is not None else 0.0),
                                scale=1.0)
                    _emit_proj(nc, psE, fw1_sb, y2_sb, DF, S1, NCH,
                               gelu_consume)

                with tc.tile_pool(name="ffn2", bufs=1) as f2p, \
                     tc.tile_pool(name="psF", bufs=2, space="PSUM") as psF:
                    fw2_sb = f2p.tile([P, DF // P, D], F32R, tag="fw2")
                    nc.sync.dma_start(fw2_sb[:], _r3(fw2T))
                    f2_ps = []
                    _emit_proj(nc, psF, fw2_sb, h_sb, D, S1, DF // P,
                               lambda m, ps: f2_ps.append(ps))
                    _emit_ln(nc, cx, psF, lambda m: f2_ps[m], y2_sb, fb2_sb,
                             yT_sb, S1, gb_sb[3])
            for m in range(NCH):
                nc.sync.dma_start(_r3(yT)[:, m, :], yT_sb[:, m, :])

    nc.finalize()
    return nc


def _to_pm(vec, cols):
    """[cols*128] vector -> [128, cols] partition-major layout."""
    return np.ascontiguousarray(vec.reshape(cols, P).T).astype(np.float32)


def kernel(**inputs):
    cords = np.asarray(inputs["cords_features"], np.float32)
    spatial = np.asarray(inputs["spatial_features"], np.float32)
    speed = np.asarray(inputs["speed_features"], np.float32)
    B = cords.shape[0]
    assert B == 8

    def g(name):
        return np.asarray(inputs[name], np.float32)

    flags = (
        not np.allclose(g("bo1"), 0), not np.allclose(g("bo2"), 0),
        not np.allclose(g("ffn_b1"), 0), not np.allclose(g("ffn_b2"), 0),
        not (np.allclose(g("ln1_g"), 1) and np.allclose(g("ln1_b"), 0)),
        not (np.allclose(g("ln2_g"), 1) and np.allclose(g("ln2_b"), 0)),
        not (np.allclose(g("ln3_g"), 1) and np.allclose(g("ln3_b"), 0)),
    )
    if flags not in _PROGRAM_CACHE:
        _PROGRAM_CACHE[flags] = _build_program(flags)
    nc = _PROGRAM_CACHE[flags]

    shared = {
        "wq1T": np.ascontiguousarray(g("wq1").T),
        "wk1T": np.ascontiguousarray(g("wk1").T),
        "wv1T": np.ascontiguousarray(g("wv1").T),
        "wo1T": np.ascontiguousarray(g("wo1").T),
        "wq2T": np.ascontiguousarray(g("wq2").T),
        "wk2T": np.ascontiguousarray(g("wk2").T),
        "wv2T": np.ascontiguousarray(g("wv2").T),
        "wo2T": np.ascontiguousarray(g("wo2").T),
        "fw1T": np.ascontiguousarray(g("ffn_w1").T),
        "fw2T": np.ascontiguousarray(g("ffn_w2").T),
        "onesd": np.ones((P, 1), np.float32),
        "vones": np.ones((P, H), np.float32),
    }
    use_bo1, use_bo2, use_fb1, use_fb2, use_g1, use_g2, use_g3 = flags
    if use_bo1:
        shared["bo1"] = _to_pm(g("bo1"), NCH)
    if use_bo2:
        shared["bo2"] = _to_pm(g("bo2"), NCH)
    if use_fb1:
        shared["fb1"] = _to_pm(g("ffn_b1"), DF // P)
    if use_fb2:
        shared["fb2"] = _to_pm(g("ffn_b2"), NCH)
    for i, use in ((1, use_g1), (2, use_g2), (3, use_g3)):
        if use:
            shared[f"g{i}"] = _to_pm(g(f"ln{i}_g"), NCH)
            shared[f"b{i}"] = _to_pm(g(f"ln{i}_b"), NCH)

    in_maps = []
    for b in range(B):
        m = dict(shared)
        m["x1T"] = np.ascontiguousarray(cords[b].T)
        m["x2T"] = np.ascontiguousarray(spatial[b].T)
        m["x3T"] = np.ascontiguousarray(speed[b].T)
        in_maps.append(m)

    global _LAST_IN_MAPS
    _LAST_IN_MAPS = in_maps
    res = run_bass_kernel_spmd(nc, in_maps, core_ids=list(range(B)))
    out = np.stack([res.results[b]["yT"].T for b in range(B)], axis=0)
    return np.ascontiguousarray(out.astype(np.float32))

